# revision 1
# baseline (speedup 1.0000x reference)
"""Trainium2 Bass kernel for the CRF loss (nn_CRFLayer_83270825935102).

Segmented rank-1 forward algorithm. Full inputs in, full output out;
data-parallel over the batch across 8 NeuronCores (64 rows each).

Per core the T=1024 forward recursion is split into S=16 segments glued
with rank-1 transfer-operator approximations (truncation error
~(lambda2/lambda1)^64 ~ 1e-16, validated < 3.2e-4 per row): chains
h (exact fwd, seg 0), a1..a14 (fwd from ones), g (exact bwd, seg 15)
run 64 serial slots CONCURRENTLY, plus fourteen 8-step backward probes
u1..u14 giving left vectors whose scale cancels between numerator and
denominator joins. All chains share one instruction shape
    psum = blockdiag(expA, expA^T) @ state ; state' = psum * x_slab
with 2 chains per 128-partition instruction and 4 pair-blocks per 256-col
DVE mul (stitched 3D access patterns), so per-step engine overheads are
amortized 8x and the serial-latency wall drops from 512 round trips
(the meet-in-the-middle baseline) to 64.

Emissions arrive host-prepared as bf16 K-major (emT[k,t,b] / emR
reversed, zero-padded to 64 partitions) so no PE transposes are needed
and the matmul weights are loaded once. The gold emission term is
gpsimd multiply against a host-built one-hot + ACT accum-copy reduce
(keeps DVE free). Norm snapshots every ~16 slots keep bf16 in range;
their exact logs and the join logs are taken in bulk Ln instructions at
the end and assembled on the host together with the tiny
start/end/transition gold glue (index math on tags only).
"""
import numpy as np

K = 48
BL = 64
N_CORES = 8
T = 1024
S = 16
L = T // S           # 64 slots
TAU = 8
CSH = 4.5
CHB = 14             # phase-B slots per x-chunk
NCH = 4              # phase-B chunks
NORM_SLOTS = (24, 40, 56)
GOLD_CHUNKS = 16
HI = 64
NPAIR_A = 15         # (h,g) + (a_p, probe_p) p=1..14
NPAIR_B = 8          # (h,g) + (a_odd, a_even) x7
NJ = 15              # joins
ND = 14              # denominators


def build_nc():
    import concourse.bass as bass
    import concourse.bacc as bacc
    import concourse.mybir as mybir
    import concourse.tile as tile
    import ml_dtypes

    f32 = mybir.dt.float32
    bf16 = mybir.dt.bfloat16
    AF = mybir.ActivationFunctionType
    AX = mybir.AxisListType

    nc = bacc.Bacc("TRN2")

    emT_d = nc.dram_tensor("emT", [64, T, BL], bf16, kind="ExternalInput")
    emR_d = nc.dram_tensor("emR", [64, T, BL], bf16, kind="ExternalInput")
    embm_d = nc.dram_tensor("embm", [128, (T // 2) * K], bf16,
                            kind="ExternalInput")
    oneh_d = nc.dram_tensor("onehot", [128, (T // 2) * K], bf16,
                            kind="ExternalInput")
    trans_d = nc.dram_tensor("transitions", [K, K], f32, kind="ExternalInput")
    start_d = nc.dram_tensor("start_transitions", [K], f32,
                             kind="ExternalInput")
    end_d = nc.dram_tensor("end_transitions", [K], f32, kind="ExternalInput")

    out_lnj_d = nc.dram_tensor("out_lnj", [(NJ + ND) * 64], f32,
                               kind="ExternalOutput")
    out_led_d = nc.dram_tensor("out_led", [2, len(NORM_SLOTS) * 512], f32,
                               kind="ExternalOutput")
    out_gold_d = nc.dram_tensor("out_gold", [128], f32, kind="ExternalOutput")

    ident_d = nc.inline_tensor(np.eye(64, dtype=np.float32), name="ident64")
    _shI = np.zeros((112, 128), dtype=ml_dtypes.bfloat16)
    for j in range(K):
        _shI[j, HI + j] = 1.0
    shI_d = nc.inline_tensor(_shI, name="shI")
    _ps = np.zeros((112, 2), dtype=ml_dtypes.bfloat16)
    _ps[0:K, 0] = 1.0
    _ps[HI:HI + K, 1] = 1.0
    pat_sum_d = nc.inline_tensor(_ps, name="pat_sum")
    _pb = np.zeros((2, 128), dtype=np.float32)
    _pb[0, 0:K] = 1.0
    _pb[1, HI:HI + K] = 1.0
    pat_bc_d = nc.inline_tensor(_pb, name="pat_bc")

    lo = [s * L for s in range(S)]
    SLABA = TAU + 1          # phase-A slabs per pair (9)
    NBQ = [4, 4, 4, 3]       # pairs per phase-A quad tile

    with tile.TileContext(nc) as tc:
        with (
            tc.tile_pool(name="singles", bufs=1) as singles,
            tc.tile_pool(name="state", bufs=3) as spool,
            tc.tile_pool(name="xA", bufs=4) as xApool,
            tc.tile_pool(name="rawA", bufs=4) as rawApool,
            tc.tile_pool(name="xB", bufs=6) as xBpool,
            tc.tile_pool(name="rawB", bufs=3) as rawBpool,
            tc.tile_pool(name="goldraw", bufs=8) as goldraw,
            tc.tile_pool(name="goldsel", bufs=2) as goldsel,
            tc.tile_pool(name="work", bufs=4) as work,
            tc.tile_pool(name="ps_mm", bufs=5, space="PSUM") as ps_mm,
            tc.tile_pool(name="ps_small", bufs=1, space="PSUM") as ps_small,
            tc.tile_pool(name="ps_bc", bufs=1, space="PSUM") as ps_bcp,
            tc.tile_pool(name="ps_misc", bufs=1, space="PSUM") as ps_misc,
        ):
            # ------------- prologue: input DMAs first (overlap) ----------
            SLABA_ = TAU + 1
            rawA_tiles = []
            for q in range(4):
                nb = NBQ[q]
                W = SLABA_ * BL
                raw = rawApool.tile([128, nb * W], bf16, tag="rawA")
                for b in range(nb):
                    p = 4 * q + b
                    nc.sync.dma_start(
                        out=raw[0:64, b * W:b * W + W],
                        in_=emT_d[:, lo[p]:lo[p] + SLABA_, :])
                    if p == 0:
                        nc.vector.memset(raw[64:128, 0:BL], 0.0)
                        nc.sync.dma_start(
                            out=raw[64:128, BL:W],
                            in_=emR_d[:, 0:TAU, :])
                    else:
                        r0 = 1023 - lo[p] - TAU
                        nc.sync.dma_start(
                            out=raw[64:128, b * W:b * W + W],
                            in_=emR_d[:, r0:r0 + SLABA_, :])
                rawA_tiles.append(raw)

            # ---------------- constants ----------------
            ident = singles.tile([64, 64], f32, tag="ident")
            nc.sync.dma_start(out=ident, in_=ident_d[:, :])
            trans_sb = singles.tile([K, K], f32, tag="trans")
            nc.sync.dma_start(out=trans_sb, in_=trans_d[:, :])
            startv = singles.tile([K, 1], f32, tag="startv")
            nc.sync.dma_start(out=startv, in_=start_d[:])
            end_hi = singles.tile([128, 1], f32, tag="endhi")
            nc.sync.dma_start(out=end_hi[HI:HI + K, 0:1], in_=end_d[:])
            shI = singles.tile([112, 128], bf16, tag="shI")
            nc.sync.dma_start(out=shI, in_=shI_d[:, :])
            pat_sum = singles.tile([112, 2], bf16, tag="patsum")
            nc.sync.dma_start(out=pat_sum, in_=pat_sum_d[:, :])
            pat_bc = singles.tile([2, 128], f32, tag="patbc")
            nc.sync.dma_start(out=pat_bc, in_=pat_bc_d[:, :])

            bias_c = singles.tile([112, 1], f32, tag="biasc")
            nc.vector.memset(bias_c, -CSH)
            expstart = singles.tile([K, 1], f32, tag="expstart")
            nc.scalar.activation(expstart, startv, AF.Exp)
            expend_hi = singles.tile([128, 1], f32, tag="expendhi")
            nc.scalar.activation(expend_hi[HI:HI + K, 0:1],
                                 end_hi[HI:HI + K, 0:1], AF.Exp)

            trans_pad = singles.tile([K, HI + K], f32, tag="transpad")
            nc.vector.memset(trans_pad[:, 0:HI], 0.0)
            nc.vector.tensor_copy(trans_pad[:, HI:HI + K], trans_sb)
            ps_tT = ps_misc.tile([112, 64], f32, tag="ps_msc")
            nc.tensor.transpose(ps_tT[0:112, 0:K], trans_pad, ident[0:K, 0:K])

            lhsT_fb = singles.tile([112, 128], bf16, tag="lhsTfb")
            nc.vector.memset(lhsT_fb, 0.0)
            nc.scalar.activation(lhsT_fb[0:K, 0:K], trans_sb, AF.Exp,
                                 bias=bias_c[0:K, 0:1])
            nc.scalar.activation(lhsT_fb[HI:HI + K, HI:HI + K],
                                 ps_tT[HI:HI + K, 0:K], AF.Exp,
                                 bias=bias_c[HI:HI + K, 0:1])
            lhsT_lo = singles.tile([112, 128], bf16, tag="lhsTlo")
            nc.vector.memset(lhsT_lo, 0.0)
            nc.scalar.activation(lhsT_lo[0:K, 0:K], trans_sb, AF.Exp,
                                 bias=bias_c[0:K, 0:1])
            lhsT_sh = singles.tile([112, 128], bf16, tag="lhsTsh")
            nc.vector.memset(lhsT_sh, 0.0)
            nc.scalar.activation(lhsT_sh[0:K, HI:HI + K], trans_sb, AF.Exp,
                                 bias=bias_c[0:K, 0:1])

            ones_red = singles.tile([128, 1], bf16, tag="onesred")
            nc.vector.memset(ones_red, 0.0)
            nc.vector.memset(ones_red[HI:HI + K, 0:1], 1.0)

            ledger = singles.tile([2, len(NORM_SLOTS) * 512], f32,
                                  tag="ledger")
            products = singles.tile([128, (NJ + ND) * 64], bf16,
                                    tag="products")
            gold_acc0 = work.tile([128, 1], f32, tag="gacc")
            nc.vector.memset(gold_acc0, 0.0)
            gold_acc = [gold_acc0]

            # ---------------- helpers ----------------
            def blkN(tile_like, col_off, bstride, nb, parts=112, p0=0):
                base = tile_like[p0:p0 + parts, :]
                return bass.AP(
                    tensor=base.tensor, offset=base.offset + col_off,
                    ap=[list(base.ap[0]), [bstride, nb], [1, BL]])

            # ---------------- phase A x-chunks ----------------
            # tile q: nb pair-blocks, block stride SLABA*64; slab s at s*64.
            def expA_range(xg, raw, s0, s1, nb, W):
                if s1 - s0 == 1:
                    nc.scalar.activation(
                        blkN(xg, s0 * BL, W, nb, parts=128),
                        blkN(raw, s0 * BL, W, nb, parts=128), AF.Exp)
                    return
                nc.scalar.activation(
                    bass.AP(tensor=xg[:, :].tensor,
                            offset=xg[:, :].offset + s0 * BL,
                            ap=[list(xg[:, :].ap[0]), [W, nb],
                                [1, (s1 - s0) * BL]]),
                    bass.AP(tensor=raw[:, :].tensor,
                            offset=raw[:, :].offset + s0 * BL,
                            ap=[list(raw[:, :].ap[0]), [W, nb],
                                [1, (s1 - s0) * BL]]), AF.Exp)

            xA = []
            for q in range(4):
                nb = NBQ[q]
                W = SLABA * BL
                raw = rawA_tiles[q]
                xg = xApool.tile([128, nb * W], bf16, tag="xA")
                # split exp so init + early slots start before full chunk done
                expA_range(xg, raw, 0, 1, nb, W)
                expA_range(xg, raw, 1, 2, nb, W)
                xA.append((xg, raw))
            for q in range(4):
                nb = NBQ[q]
                W = SLABA * BL
                xg, raw = xA[q]
                expA_range(xg, raw, 2, 5, nb, W)
            for q in range(4):
                nb = NBQ[q]
                W = SLABA * BL
                xg, raw = xA[q]
                expA_range(xg, raw, 5, SLABA, nb, W)
                # probes' last slot (TAU-1) multiplies by ones: slab TAU
                b0 = 1 if q == 0 else 0
                if nb - b0 > 0:
                    nc.vector.memset(
                        blkN(xg, b0 * W + TAU * BL, W, nb - b0, parts=64,
                             p0=64), 1.0)
            xA = [t[0] for t in xA]

            # ---------------- phase B x-chunks ----------------
            WB = CHB * BL
            xB = {}

            def load_chunk_B(q, c):
                raw = rawBpool.tile([128, 4 * WB], bf16, tag="rawB")
                for b in range(4):
                    p = 4 * q + b
                    co = b * WB
                    if p == 0:
                        nc.sync.dma_start(
                            out=raw[0:64, co:co + WB],
                            in_=emT_d[:, TAU + 1 + CHB * c:
                                      TAU + 1 + CHB * c + CHB, :])
                        nc.sync.dma_start(
                            out=raw[64:128, co:co + WB],
                            in_=emR_d[:, TAU + CHB * c:TAU + CHB * c + CHB, :])
                    else:
                        sl = 2 * p - 1
                        su = 2 * p
                        nc.sync.dma_start(
                            out=raw[0:64, co:co + WB],
                            in_=emT_d[:, lo[sl] + TAU + 1 + CHB * c:
                                      lo[sl] + TAU + 1 + CHB * c + CHB, :])
                        nc.sync.dma_start(
                            out=raw[64:128, co:co + WB],
                            in_=emT_d[:, lo[su] + TAU + 1 + CHB * c:
                                      lo[su] + TAU + 1 + CHB * c + CHB, :])
                xg = xBpool.tile([128, 4 * WB], bf16, tag="xB")
                # split exp into halves: the chain unblocks on the first
                # slabs sooner, and gold ACT copies interleave finer
                half = (CHB // 2) * BL
                for s0, s1 in ((0, half), (half, WB)):
                    nc.scalar.activation(
                        bass.AP(tensor=xg[:, :].tensor,
                                offset=xg[:, :].offset + s0,
                                ap=[list(xg[:, :].ap[0]), [WB, 4],
                                    [1, s1 - s0]]),
                        bass.AP(tensor=raw[:, :].tensor,
                                offset=raw[:, :].offset + s0,
                                ap=[list(raw[:, :].ap[0]), [WB, 4],
                                    [1, s1 - s0]]), AF.Exp)
                if q == 0 and c == NCH - 1:
                    # slot 63: g's trailing pure matmul -> ones slab
                    nc.vector.memset(xg[64:128, (CHB - 1) * BL:CHB * BL], 1.0)
                xB[(q, c)] = xg

            load_chunk_B(0, 0)
            load_chunk_B(1, 0)

            # ---------------- state init ----------------
            st0 = spool.tile([128, NPAIR_A * BL], bf16, tag="st")
            nc.vector.memset(st0, 1.0)
            nc.vector.tensor_mul(
                st0[0:K, 0:BL], xA[0][0:K, 0:BL],
                bass.AP(tensor=expstart.tensor, offset=expstart.offset,
                        ap=[list(expstart.ap[0]), [0, BL]]))
            # probe inits: state block p <- xA tile q block b slab 0 (upper)
            for q in range(4):
                nb = NBQ[q]
                b0 = 1 if q == 0 else 0
                if nb - b0 <= 0:
                    continue
                p_first = 4 * q + b0
                W = SLABA * BL
                nc.vector.tensor_copy(
                    blkN(st0, p_first * BL, BL, nb - b0, parts=K, p0=HI),
                    blkN(xA[q], b0 * W, W, nb - b0, parts=K, p0=HI))
            state = [st0]

            # ---------------- gold pieces (gpsimd + ACT reduce) ----------
            # loads prefetch ahead (goldraw bufs=8 -> 4 chunks in flight);
            # computes are paced through the slot loops.
            gw = (T // 2) * K // GOLD_CHUNKS
            gold_tiles = {}
            gold_loaded = [0]

            def load_gold():
                c = gold_loaded[0]
                if c >= GOLD_CHUNKS:
                    return
                ebm = goldraw.tile([128, gw], bf16, tag="gebm")
                nc.sync.dma_start(out=ebm,
                                  in_=embm_d[:, c * gw:(c + 1) * gw])
                ohc = goldraw.tile([128, gw], bf16, tag="goh")
                nc.sync.dma_start(out=ohc,
                                  in_=oneh_d[:, c * gw:(c + 1) * gw])
                gold_tiles[c] = (ebm, ohc)
                gold_loaded[0] = c + 1

            gold_q = []

            def gold_piece(c):
                def run(c=c):
                    ebm, ohc = gold_tiles.pop(c)
                    sel = goldsel.tile([128, gw], bf16, tag="gsel")
                    nc.gpsimd.tensor_mul(sel, ohc, ebm)
                    sel2 = goldsel.tile([128, gw], bf16, tag="gsel2")
                    part = work.tile([128, 1], f32, tag="gpart")
                    nc.scalar.activation(sel2, sel, AF.Copy,
                                         accum_out=part)
                    nacc = work.tile([128, 1], f32, tag="gacc")
                    nc.vector.tensor_add(nacc, gold_acc[0], part)
                    gold_acc[0] = nacc
                    load_gold()
                return run
            for c in range(GOLD_CHUNKS):
                gold_q.append(gold_piece(c))
            for _ in range(4):
                load_gold()

            def norm_snapshot(n):
                stn = state[0]
                ps_sum = ps_small.tile([2, 512], f32, tag="ps_sm")
                nc.tensor.matmul(ps_sum, pat_sum, stn[0:112, 0:512],
                                 start=True, stop=True)
                recip = work.tile([2, 512], f32, tag="recip")
                nc.vector.reciprocal_approx_fast(recip, ps_sum)
                snap_i = NORM_SLOTS.index(n)
                nc.vector.tensor_copy(
                    ledger[:, snap_i * 512:(snap_i + 1) * 512], recip)
                psb = ps_bcp.tile([128, 512], f32, tag="ps_bc")
                nc.tensor.matmul(psb, pat_bc, recip, start=True, stop=True)
                tgt = n + 2
                c, i = divmod(tgt - TAU, CHB)
                for q in range(2):
                    xt = xB[(q, c)]
                    nc.vector.tensor_mul(
                        blkN(xt, i * BL, WB, 4),
                        blkN(xt, i * BL, WB, 4),
                        blkN(psb, q * 256, BL, 4))

            # ---------------- phase A slots 0..TAU-1 ----------------
            col0 = [0, 256, 512, 768]
            for j in range(TAU):
                ps_q = []
                for q in range(4):
                    nb = NBQ[q]
                    ps = ps_mm.tile([128, 256], f32, tag="ps_mm")
                    nc.tensor.matmul(
                        ps[:, 0:nb * BL], lhsT_fb,
                        state[0][0:112, col0[q]:col0[q] + nb * BL],
                        start=True, stop=True)
                    ps_q.append(ps)
                stn = spool.tile([128, NPAIR_A * BL], bf16, tag="st")
                for q in range(4):
                    nb = NBQ[q]
                    W = SLABA * BL
                    nc.vector.tensor_mul(
                        blkN(stn, col0[q], BL, nb),
                        blkN(ps_q[q], 0, BL, nb),
                        blkN(xA[q], (j + 1) * BL, W, nb))
                state = [stn]
                if j == 0:
                    fexp = expend_hi[HI:HI + K, 0:1]
                    nc.vector.tensor_mul(
                        stn[HI:HI + K, 0:BL],
                        xA[0][HI:HI + K, SLABA * 0 + BL:SLABA * 0 + 2 * BL],
                        bass.AP(tensor=fexp.tensor, offset=fexp.offset,
                                ap=[list(fexp.ap[0]), [0, BL]]))
                if j in (1, 4) and gold_q:
                    gold_q.pop(0)()
                if j == 5:
                    load_chunk_B(0, 1)
                    load_chunk_B(1, 1)

            # probe saves: u1..u14 -> products cols NJ*64 ..
            nc.vector.tensor_copy(products[HI:HI + K, NJ * 64:NJ * 64 + 896],
                                  state[0][HI:HI + K, BL:NPAIR_A * BL])

            # ---------------- transition (slot TAU) ----------------
            stA = state[0]
            ps_t = []
            for g2 in range(2):
                ps = ps_mm.tile([128, 256], f32, tag="ps_mm")
                ps_t.append(ps)
            nc.tensor.matmul(ps_t[0][:, 0:64], lhsT_fb, stA[0:112, 0:64],
                             start=True, stop=True)
            for k in range(1, NPAIR_B):
                ps = ps_t[k // 4]
                co = (k % 4) * BL
                nc.tensor.matmul(ps[:, co:co + BL], lhsT_lo,
                                 stA[0:112, (2 * k - 1) * BL:2 * k * BL],
                                 start=True, stop=False)
                nc.tensor.matmul(ps[:, co:co + BL], lhsT_sh,
                                 stA[0:112, 2 * k * BL:(2 * k + 1) * BL],
                                 start=False, stop=True)
            stn = spool.tile([128, NPAIR_A * BL], bf16, tag="st")
            for q in range(2):
                nc.vector.tensor_mul(
                    blkN(stn, q * 256, BL, 4), blkN(ps_t[q], 0, BL, 4),
                    blkN(xB[(q, 0)], 0, WB, 4))
            state = [stn]

            # ---------------- phase B slots TAU+1..L-1 ----------------
            for j in range(TAU + 1, L):
                c, i = divmod(j - TAU, CHB)
                ps_q = []
                for q in range(2):
                    ps = ps_mm.tile([128, 256], f32, tag="ps_mm")
                    nc.tensor.matmul(ps, lhsT_fb,
                                     state[0][0:112, q * 256:(q + 1) * 256],
                                     start=True, stop=True)
                    ps_q.append(ps)
                stn = spool.tile([128, NPAIR_A * BL], bf16, tag="st")
                for q in range(2):
                    nc.vector.tensor_mul(
                        blkN(stn, q * 256, BL, 4), blkN(ps_q[q], 0, BL, 4),
                        blkN(xB[(q, c)], i * BL, WB, 4))
                state = [stn]
                if j % 3 == 0 and gold_q:
                    gold_q.pop(0)()
                if i == 2 and c + 2 < NCH:
                    load_chunk_B(0, c + 2)
                    load_chunk_B(1, c + 2)
                if j in NORM_SLOTS:
                    norm_snapshot(j)
            while gold_q:
                gold_q.pop(0)()

            # ---------------- epilogue: joins ----------------
            stF = state[0]
            ps_shift = ps_bcp.tile([128, 512], f32, tag="ps_bc")
            nc.tensor.matmul(ps_shift, shI, stF[0:112, 0:512],
                             start=True, stop=True)
            U = lambda c0: products[HI:HI + K, c0:c0 + BL]
            # J_s = u_s * a_{s-1}  (a_0 = h); a_odd lower (shifted),
            # a_even upper (direct). J_15 = g * a_14.
            for s in range(1, NJ):
                us = U(NJ * 64 + (s - 1) * 64)
                am1 = s - 1
                if am1 % 2 == 0:
                    # a_{even} incl h: lower chain of pair (am1//2? )
                    # h=pair0 lower; a_2k = upper of pair k... a_even:
                    # a_0=h lower pair0; a_2,a_4..: upper of pair 1..:
                    if am1 == 0:
                        src = ps_shift[HI:HI + K, 0:64]
                    else:
                        kk = am1 // 2
                        src = stF[HI:HI + K, kk * 64:kk * 64 + 64]
                else:
                    kk = (am1 + 1) // 2
                    src = ps_shift[HI:HI + K, kk * 64:kk * 64 + 64]
                nc.vector.tensor_mul(U((s - 1) * 64), us, src)
            nc.vector.tensor_mul(U((NJ - 1) * 64), stF[HI:HI + K, 0:BL],
                                 stF[HI:HI + K, 7 * 64:8 * 64])
            lnj = singles.tile([1, (NJ + ND) * 64], f32, tag="lnj")
            TOT = (NJ + ND) * 64
            off = 0
            while off < TOT:
                wdt = min(512, TOT - off)
                ps_red = ps_small.tile([1, 512], f32, tag="ps_sm")
                nc.tensor.matmul(ps_red[0:1, 0:wdt],
                                 ones_red[HI:HI + K, 0:1],
                                 products[HI:HI + K, off:off + wdt],
                                 start=True, stop=True)
                nc.scalar.activation(lnj[0:1, off:off + wdt],
                                     ps_red[0:1, 0:wdt], AF.Ln)
                off += wdt
            lnled = singles.tile([2, len(NORM_SLOTS) * 512], f32, tag="lnled")
            nc.scalar.activation(lnled, ledger, AF.Ln)

            nc.sync.dma_start(out=out_lnj_d[:], in_=lnj)
            nc.sync.dma_start(out=out_led_d[:, :], in_=lnled)
            nc.sync.dma_start(out=out_gold_d[:], in_=gold_acc[0])

    nc.finalize()
    return nc


_NC_CACHE = {}
TRACE = False
LAST_RESULT = None


def _prep_core(em_c, tags_c):
    import ml_dtypes
    bf = ml_dtypes.bfloat16
    emb = em_c.astype(bf)
    emT = np.zeros((64, T, BL), dtype=bf)
    emT[0:K] = emb.transpose(2, 1, 0)
    emR = np.zeros((64, T, BL), dtype=bf)
    emR[0:K] = emb[:, ::-1, :].transpose(2, 1, 0)
    h = T // 2
    embm = np.concatenate([emb[:, 0:h].reshape(BL, h * K),
                           emb[:, h:T].reshape(BL, h * K)], axis=0)
    oh = np.zeros((BL, T, K), dtype=bf)
    np.put_along_axis(oh, tags_c[:, :, None], np.asarray(1.0, dtype=bf), 2)
    oneh = np.concatenate([oh[:, 0:h].reshape(BL, h * K),
                           oh[:, h:T].reshape(BL, h * K)], axis=0)
    return (np.ascontiguousarray(emT), np.ascontiguousarray(emR),
            np.ascontiguousarray(embm), np.ascontiguousarray(oneh))


def kernel(emissions, transitions, start_transitions, end_transitions,
           tags, mask=None, **_):
    emissions = np.ascontiguousarray(np.asarray(emissions, dtype=np.float32))
    transitions = np.ascontiguousarray(np.asarray(transitions,
                                                  dtype=np.float32))
    start_transitions = np.ascontiguousarray(
        np.asarray(start_transitions, dtype=np.float32))
    end_transitions = np.ascontiguousarray(
        np.asarray(end_transitions, dtype=np.float32))
    tags_i = np.ascontiguousarray(np.asarray(tags).astype(np.int64))

    B, Tt, Kk = emissions.shape
    assert Kk == K and B == N_CORES * BL and Tt == T

    from concourse import bass_utils
    if T not in _NC_CACHE:
        _NC_CACHE[T] = build_nc()
    nc = _NC_CACHE[T]

    in_maps = []
    for c in range(N_CORES):
        sl = slice(c * BL, (c + 1) * BL)
        emT, emR, embm, oneh = _prep_core(emissions[sl], tags_i[sl])
        in_maps.append({
            "emT": emT, "emR": emR, "embm": embm, "onehot": oneh,
            "transitions": transitions,
            "start_transitions": start_transitions,
            "end_transitions": end_transitions,
        })
    global LAST_RESULT
    res = bass_utils.run_bass_kernel_spmd(nc, in_maps, list(range(N_CORES)),
                                          trace=TRACE)
    LAST_RESULT = res

    b = np.arange(BL)
    loss_rows = []
    for c in range(N_CORES):
        r = res.results[c]
        lnj = r["out_lnj"].astype(np.float64)
        led = r["out_led"].astype(np.float64)
        gold_dev = r["out_gold"].astype(np.float64)
        logZ = np.zeros(BL)
        for jj in range(NJ):
            logZ += lnj[jj * 64 + b]
        for ii in range(ND):
            logZ -= lnj[(NJ + ii) * 64 + b]
        for s in range(len(NORM_SLOTS)):
            for hh in range(2):
                for blk in range(8):
                    logZ -= led[hh, s * 512 + blk * 64 + b]
        logZ += CSH * (T - 1)
        gold_em = gold_dev[b] + gold_dev[64 + b]
        loss_rows.append(logZ - gold_em)
    loss_rows = np.concatenate(loss_rows)

    glue = transitions.astype(np.float64)[tags_i[:, :-1], tags_i[:, 1:]].sum(1)
    glue += start_transitions.astype(np.float64)[tags_i[:, 0]]
    glue += end_transitions.astype(np.float64)[tags_i[:, -1]]
    loss = (loss_rows - glue).mean()
    return np.float32(loss)



# revision 4
# speedup vs baseline: 1.2741x; 1.2741x over previous
"""Trainium2 Bass kernel for the CRF loss (nn_CRFLayer_83270825935102).

Segmented rank-1 forward algorithm. Full inputs in, full output out;
data-parallel over the batch across 8 NeuronCores (64 rows each).

Per core the T=1024 forward recursion is split into S=16 segments glued
with rank-1 transfer-operator approximations: chains h (exact fwd,
seg 0), a1..a14 (fwd from ones), g (exact bwd, seg 15) run 64 serial
slots CONCURRENTLY, plus fourteen 8-step backward probes u1..u14 giving
left vectors whose scale cancels between numerator and denominator
joins. All chains share one instruction shape
    psum = blockdiag(expA, expA^T) @ state ; state' = psum * x_slab
with 2 chains per 128-partition instruction and 4 pair-blocks per 256-col
DVE mul (stitched 3D access patterns).

The gold score (start/end/transition/emission terms, index math on tags)
is computed on the host: it is pure gather work, far cheaper on the host
than streaming a one-hot through the device. Device handles only the
forward (partition-function) recursion. Emissions arrive host-prepared
as bf16 K-major (emT[k,t,b] / emR reversed, zero-padded to 64
partitions); all constant matrices (exp(transitions - CSH) variants,
reduction patterns) are host-precomputed and shipped in two packed
DMAs. Emission DMAs are batched 3-4 pair-blocks per trigger via strided
access patterns to keep the sync queue short. Norm snapshots every ~16
slots keep bf16 in range; their exact logs and the join logs are taken
in bulk Ln instructions at the end and assembled on the host.
"""
import numpy as np

K = 48
BL = 64
N_CORES = 8
T = 1024
S = 16
L = T // S           # 64 slots
TAU = 8
CSH = 4.5
CHB = 14             # phase-B slots per x-chunk
NCH = 4              # phase-B chunks
NORM_SLOTS = (24, 40, 56)
HI = 64
NPAIR_A = 15         # (h,g) + (a_p, probe_p) p=1..14
NPAIR_B = 8          # (h,g) + (a_odd, a_even) x7
NJ = 15              # joins
ND = 14              # denominators


def build_nc():
    import concourse.bass as bass
    import concourse.bacc as bacc
    import concourse.mybir as mybir
    import concourse.tile as tile

    f32 = mybir.dt.float32
    bf16 = mybir.dt.bfloat16
    AF = mybir.ActivationFunctionType

    nc = bacc.Bacc("TRN2")

    emT_d = nc.dram_tensor("emT", [64, T, BL], bf16, kind="ExternalInput")
    emR_d = nc.dram_tensor("emR", [64, T, BL], bf16, kind="ExternalInput")
    # packed constants: [128, 518] bf16:
    #   0:128 lhsT_fb | 128:256 lhsT_lo | 256:384 lhsT_sh | 384:512 shI
    #   512:514 pat_sum | 514 ones_red | 515 (pad)
    cb_d = nc.dram_tensor("cb", [128, 516], bf16, kind="ExternalInput")
    # f32 pack: [128, 2]: col0 expstart (0:48), col1 expend_hi (64:112)
    cf_d = nc.dram_tensor("cf", [128, 2], f32, kind="ExternalInput")
    patbc_d = nc.dram_tensor("patbc", [2, 128], f32, kind="ExternalInput")

    # merged output: row0 cols 0:1856 = lnj; rows 0:2 cols 1856:3392 = led
    out_d = nc.dram_tensor("out", [2, 3392], f32, kind="ExternalOutput")

    lo = [s * L for s in range(S)]
    SLABA = TAU + 1          # phase-A slabs per pair (9)
    NBQ = [4, 4, 4, 3]       # pairs per phase-A quad tile

    with tile.TileContext(nc) as tc:
        with (
            tc.tile_pool(name="singles", bufs=1) as singles,
            tc.tile_pool(name="state", bufs=3) as spool,
            tc.tile_pool(name="xA", bufs=4) as xApool,
            tc.tile_pool(name="rawA", bufs=4) as rawApool,
            tc.tile_pool(name="xB", bufs=6) as xBpool,
            tc.tile_pool(name="rawB", bufs=3) as rawBpool,
            tc.tile_pool(name="work", bufs=4) as work,
            tc.tile_pool(name="ps_mm", bufs=5, space="PSUM") as ps_mm,
            tc.tile_pool(name="ps_small", bufs=1, space="PSUM") as ps_small,
            tc.tile_pool(name="ps_bc", bufs=1, space="PSUM") as ps_bcp,
        ):
            # ------------- prologue: input DMAs first (overlap) ----------
            SLABA_ = TAU + 1
            W = SLABA_ * BL
            rawA_tiles = []
            for q in range(4):
                nb = NBQ[q]
                raw = rawApool.tile([128, nb * W], bf16, tag="rawA")
                rawA_tiles.append(raw)
            for q in range(4):
                nb = NBQ[q]
                raw = rawA_tiles[q]
                # upper: emT slabs lo[p]..lo[p]+9, p=4q+b -> src stride 4096
                up = raw[0:64, :]
                nc.sync.dma_start(
                    out=bass.AP(tensor=up.tensor, offset=up.offset,
                                ap=[list(up.ap[0]), [W, nb], [1, W]]),
                    in_=bass.AP(tensor=emT_d[:, :, :].tensor,
                                offset=emT_d[:, :, :].offset + 4 * q * 4096,
                                ap=[[T * BL, 64], [4096, nb], [1, W]]))
                # lower: emR slabs r0..r0+9, r0 = (1015 - 64p) -> stride -4096
                b0 = 1 if q == 0 else 0
                if q == 0:
                    nc.vector.memset(raw[64:128, 0:BL], 0.0)
                    nc.sync.dma_start(
                        out=raw[64:128, BL:W],
                        in_=emR_d[:, 0:TAU, :])
                if nb - b0 > 0:
                    p_first = 4 * q + b0
                    dn = raw[64:128, :]
                    nc.sync.dma_start(
                        out=bass.AP(tensor=dn.tensor,
                                    offset=dn.offset + b0 * W,
                                    ap=[list(dn.ap[0]), [W, nb - b0], [1, W]]),
                        in_=bass.AP(
                            tensor=emR_d[:, :, :].tensor,
                            offset=emR_d[:, :, :].offset
                            + (1015 - 64 * p_first) * 64,
                            ap=[[T * BL, 64], [-4096, nb - b0], [1, W]]))

            # ---------------- constants (2 packed DMAs + patbc) ----------
            cb = singles.tile([128, 516], bf16, tag="cb")
            nc.sync.dma_start(out=cb, in_=cb_d[:, :])
            cf = singles.tile([128, 2], f32, tag="cf")
            nc.sync.dma_start(out=cf, in_=cf_d[:, :])
            pat_bc = singles.tile([2, 128], f32, tag="patbc")
            nc.sync.dma_start(out=pat_bc, in_=patbc_d[:, :])

            lhsT_fb = cb[0:112, 0:128]
            lhsT_lo = cb[0:112, 128:256]
            lhsT_sh = cb[0:112, 256:384]
            shI = cb[0:112, 384:512]
            pat_sum = cb[0:112, 512:514]
            ones_red = cb[0:128, 514:515]
            expstart = cf[0:K, 0:1]
            expend_hi = cf[0:128, 1:2]

            ledger = singles.tile([2, len(NORM_SLOTS) * 512], f32,
                                  tag="ledger")
            products = singles.tile([128, (NJ + ND) * 64], bf16,
                                    tag="products")

            # ---------------- helpers ----------------
            def blkN(tile_like, col_off, bstride, nb, parts=112, p0=0):
                base = tile_like[p0:p0 + parts, :]
                return bass.AP(
                    tensor=base.tensor, offset=base.offset + col_off,
                    ap=[list(base.ap[0]), [bstride, nb], [1, BL]])

            # ---------------- phase A x-chunks ----------------
            def expA_range(xg, raw, s0, s1, nb):
                nc.scalar.activation(
                    bass.AP(tensor=xg[:, :].tensor,
                            offset=xg[:, :].offset + s0 * BL,
                            ap=[list(xg[:, :].ap[0]), [W, nb],
                                [1, (s1 - s0) * BL]]),
                    bass.AP(tensor=raw[:, :].tensor,
                            offset=raw[:, :].offset + s0 * BL,
                            ap=[list(raw[:, :].ap[0]), [W, nb],
                                [1, (s1 - s0) * BL]]), AF.Exp)

            xA = []
            for q in range(4):
                nb = NBQ[q]
                raw = rawA_tiles[q]
                xg = xApool.tile([128, nb * W], bf16, tag="xA")
                # split exp so init + early slots start before full chunk done
                expA_range(xg, raw, 0, 1, nb)
                expA_range(xg, raw, 1, 2, nb)
                xA.append((xg, raw))
            for q in range(4):
                expA_range(xA[q][0], xA[q][1], 2, 5, NBQ[q])
            for q in range(4):
                nb = NBQ[q]
                xg, raw = xA[q]
                expA_range(xg, raw, 5, SLABA, nb)
                # probes' last slot (TAU-1) multiplies by ones: slab TAU
                b0 = 1 if q == 0 else 0
                if nb - b0 > 0:
                    nc.vector.memset(
                        blkN(xg, b0 * W + TAU * BL, W, nb - b0, parts=64,
                             p0=64), 1.0)
            xA = [t[0] for t in xA]

            # ---------------- phase B x-chunks ----------------
            WB = CHB * BL
            xB = {}

            def load_chunk_B(q, c):
                raw = rawBpool.tile([128, 4 * WB], bf16, tag="rawB")
                # upper: p=4q+b, seg sl=2p-1 (b>=1 or q>0): src stride 8192
                # q=0 b=0 special: emT slabs 9+14c / emR slabs 8+14c
                if q == 0:
                    nc.sync.dma_start(
                        out=raw[0:64, 0:WB],
                        in_=emT_d[:, TAU + 1 + CHB * c:
                                  TAU + 1 + CHB * c + CHB, :])
                    nc.sync.dma_start(
                        out=raw[64:128, 0:WB],
                        in_=emR_d[:, TAU + CHB * c:TAU + CHB * c + CHB, :])
                    bb0 = 1
                else:
                    bb0 = 0
                nbb = 4 - bb0
                p_first = 4 * q + bb0
                for half, seg0 in ((0, 2 * p_first - 1), (64, 2 * p_first)):
                    hr = raw[half:half + 64, :]
                    nc.sync.dma_start(
                        out=bass.AP(tensor=hr.tensor,
                                    offset=hr.offset + bb0 * WB,
                                    ap=[list(hr.ap[0]), [WB, nbb], [1, WB]]),
                        in_=bass.AP(
                            tensor=emT_d[:, :, :].tensor,
                            offset=emT_d[:, :, :].offset
                            + (seg0 * 64 + TAU + 1 + CHB * c) * 64,
                            ap=[[T * BL, 64], [8192, nbb], [1, WB]]))
                xg = xBpool.tile([128, 4 * WB], bf16, tag="xB")
                half = (CHB // 2) * BL
                for s0, s1 in ((0, half), (half, WB)):
                    nc.scalar.activation(
                        bass.AP(tensor=xg[:, :].tensor,
                                offset=xg[:, :].offset + s0,
                                ap=[list(xg[:, :].ap[0]), [WB, 4],
                                    [1, s1 - s0]]),
                        bass.AP(tensor=raw[:, :].tensor,
                                offset=raw[:, :].offset + s0,
                                ap=[list(raw[:, :].ap[0]), [WB, 4],
                                    [1, s1 - s0]]), AF.Exp)
                if q == 0 and c == NCH - 1:
                    # slot 63: g's trailing pure matmul -> ones slab
                    nc.vector.memset(xg[64:128, (CHB - 1) * BL:CHB * BL], 1.0)
                xB[(q, c)] = xg

            load_chunk_B(0, 0)
            load_chunk_B(1, 0)

            # ---------------- state init ----------------
            st0 = spool.tile([128, NPAIR_A * BL], bf16, tag="st")
            nc.vector.memset(st0, 1.0)
            nc.vector.tensor_mul(
                st0[0:K, 0:BL], xA[0][0:K, 0:BL],
                bass.AP(tensor=expstart.tensor, offset=expstart.offset,
                        ap=[list(expstart.ap[0]), [0, BL]]))
            # probe inits: state block p <- xA tile q block b slab 0 (upper)
            for q in range(4):
                nb = NBQ[q]
                b0 = 1 if q == 0 else 0
                if nb - b0 <= 0:
                    continue
                p_first = 4 * q + b0
                nc.vector.tensor_copy(
                    blkN(st0, p_first * BL, BL, nb - b0, parts=K, p0=HI),
                    blkN(xA[q], b0 * W, W, nb - b0, parts=K, p0=HI))
            state = [st0]

            def norm_snapshot(n):
                stn = state[0]
                ps_sum = ps_small.tile([2, 512], f32, tag="ps_sm")
                nc.tensor.matmul(ps_sum, pat_sum, stn[0:112, 0:512],
                                 start=True, stop=True)
                recip = work.tile([2, 512], f32, tag="recip")
                nc.vector.reciprocal_approx_fast(recip, ps_sum)
                snap_i = NORM_SLOTS.index(n)
                nc.vector.tensor_copy(
                    ledger[:, snap_i * 512:(snap_i + 1) * 512], recip)
                psb = ps_bcp.tile([128, 512], f32, tag="ps_bc")
                nc.tensor.matmul(psb, pat_bc, recip, start=True, stop=True)
                tgt = n + 2
                c, i = divmod(tgt - TAU, CHB)
                for q in range(2):
                    xt = xB[(q, c)]
                    nc.vector.tensor_mul(
                        blkN(xt, i * BL, WB, 4),
                        blkN(xt, i * BL, WB, 4),
                        blkN(psb, q * 256, BL, 4))

            # ---------------- phase A slots 0..TAU-1 ----------------
            col0 = [0, 256, 512, 768]
            for j in range(TAU):
                ps_q = []
                for q in range(4):
                    nb = NBQ[q]
                    ps = ps_mm.tile([128, 256], f32, tag="ps_mm")
                    nc.tensor.matmul(
                        ps[:, 0:nb * BL], lhsT_fb,
                        state[0][0:112, col0[q]:col0[q] + nb * BL],
                        start=True, stop=True)
                    ps_q.append(ps)
                stn = spool.tile([128, NPAIR_A * BL], bf16, tag="st")
                for q in range(4):
                    nb = NBQ[q]
                    nc.vector.tensor_mul(
                        blkN(stn, col0[q], BL, nb),
                        blkN(ps_q[q], 0, BL, nb),
                        blkN(xA[q], (j + 1) * BL, W, nb))
                state = [stn]
                if j == 0:
                    fexp = expend_hi[HI:HI + K, 0:1]
                    nc.vector.tensor_mul(
                        stn[HI:HI + K, 0:BL],
                        xA[0][HI:HI + K, BL:2 * BL],
                        bass.AP(tensor=fexp.tensor, offset=fexp.offset,
                                ap=[list(fexp.ap[0]), [0, BL]]))
                if j == 5:
                    load_chunk_B(0, 1)
                    load_chunk_B(1, 1)

            # probe saves: u1..u14 -> products cols NJ*64 ..
            nc.vector.tensor_copy(products[HI:HI + K, NJ * 64:NJ * 64 + 896],
                                  state[0][HI:HI + K, BL:NPAIR_A * BL])

            # ---------------- transition (slot TAU) ----------------
            stA = state[0]
            ps_t = []
            for g2 in range(2):
                ps = ps_mm.tile([128, 256], f32, tag="ps_mm")
                ps_t.append(ps)
            nc.tensor.matmul(ps_t[0][:, 0:64], lhsT_fb, stA[0:112, 0:64],
                             start=True, stop=True)
            for k in range(1, NPAIR_B):
                ps = ps_t[k // 4]
                co = (k % 4) * BL
                nc.tensor.matmul(ps[:, co:co + BL], lhsT_lo,
                                 stA[0:112, (2 * k - 1) * BL:2 * k * BL],
                                 start=True, stop=False)
                nc.tensor.matmul(ps[:, co:co + BL], lhsT_sh,
                                 stA[0:112, 2 * k * BL:(2 * k + 1) * BL],
                                 start=False, stop=True)
            stn = spool.tile([128, NPAIR_A * BL], bf16, tag="st")
            for q in range(2):
                nc.vector.tensor_mul(
                    blkN(stn, q * 256, BL, 4), blkN(ps_t[q], 0, BL, 4),
                    blkN(xB[(q, 0)], 0, WB, 4))
            state = [stn]

            # ---------------- phase B slots TAU+1..L-1 ----------------
            for j in range(TAU + 1, L):
                c, i = divmod(j - TAU, CHB)
                ps_q = []
                for q in range(2):
                    ps = ps_mm.tile([128, 256], f32, tag="ps_mm")
                    nc.tensor.matmul(ps, lhsT_fb,
                                     state[0][0:112, q * 256:(q + 1) * 256],
                                     start=True, stop=True)
                    ps_q.append(ps)
                stn = spool.tile([128, NPAIR_A * BL], bf16, tag="st")
                for q in range(2):
                    nc.vector.tensor_mul(
                        blkN(stn, q * 256, BL, 4), blkN(ps_q[q], 0, BL, 4),
                        blkN(xB[(q, c)], i * BL, WB, 4))
                state = [stn]
                if i == 2 and c + 2 < NCH:
                    load_chunk_B(0, c + 2)
                    load_chunk_B(1, c + 2)
                if j in NORM_SLOTS:
                    norm_snapshot(j)

            # ---------------- epilogue: joins ----------------
            stF = state[0]
            ps_shift = ps_bcp.tile([128, 512], f32, tag="ps_bc")
            nc.tensor.matmul(ps_shift, shI, stF[0:112, 0:512],
                             start=True, stop=True)
            U = lambda c0: products[HI:HI + K, c0:c0 + BL]
            # J_s = u_s * a_{s-1}  (a_0 = h); a_odd lower (shifted),
            # a_even upper (direct). J_15 = g * a_14.
            for s in range(1, NJ):
                us = U(NJ * 64 + (s - 1) * 64)
                am1 = s - 1
                if am1 % 2 == 0:
                    if am1 == 0:
                        src = ps_shift[HI:HI + K, 0:64]
                    else:
                        kk = am1 // 2
                        src = stF[HI:HI + K, kk * 64:kk * 64 + 64]
                else:
                    kk = (am1 + 1) // 2
                    src = ps_shift[HI:HI + K, kk * 64:kk * 64 + 64]
                nc.vector.tensor_mul(U((s - 1) * 64), us, src)
            nc.vector.tensor_mul(U((NJ - 1) * 64), stF[HI:HI + K, 0:BL],
                                 stF[HI:HI + K, 7 * 64:8 * 64])
            outbuf = singles.tile([2, 3392], f32, tag="outbuf")
            lnj = outbuf[0:1, 0:1856]
            TOT = (NJ + ND) * 64
            off = 0
            while off < TOT:
                wdt = min(512, TOT - off)
                ps_red = ps_small.tile([1, 512], f32, tag="ps_sm")
                nc.tensor.matmul(ps_red[0:1, 0:wdt],
                                 ones_red[HI:HI + K, 0:1],
                                 products[HI:HI + K, off:off + wdt],
                                 start=True, stop=True)
                nc.scalar.activation(lnj[0:1, off:off + wdt],
                                     ps_red[0:1, 0:wdt], AF.Ln)
                off += wdt
            nc.scalar.activation(outbuf[0:2, 1856:3392], ledger, AF.Ln)

            nc.sync.dma_start(out=out_d[:, :], in_=outbuf)

    nc.finalize()
    return nc


_NC_CACHE = {}
TRACE = False
LAST_RESULT = None


def _prep_core(em_c):
    import ml_dtypes
    bf = ml_dtypes.bfloat16
    emb = em_c.astype(bf)
    emT = np.zeros((64, T, BL), dtype=bf)
    emT[0:K] = emb.transpose(2, 1, 0)
    emR = np.zeros((64, T, BL), dtype=bf)
    emR[0:K] = emb[:, ::-1, :].transpose(2, 1, 0)
    return np.ascontiguousarray(emT), np.ascontiguousarray(emR)


def _build_const_arrays(transitions, start_transitions, end_transitions):
    import ml_dtypes
    bf = ml_dtypes.bfloat16
    trans = transitions.astype(np.float64)
    expA = np.exp(trans - CSH)
    cb = np.zeros((128, 516), dtype=bf)
    # lhsT_fb: fwd block [0:48,0:48], bwd(transpose) block [64:112,64:112]
    cb[0:K, 0:K] = expA.astype(bf)
    cb[HI:HI + K, HI:HI + K] = expA.T.astype(bf)
    # lhsT_lo: fwd block only at [0:48, 128+0:128+48]
    cb[0:K, 128:128 + K] = expA.astype(bf)
    # lhsT_sh: fwd block shifted to out partitions 64:112
    cb[0:K, 256 + HI:256 + HI + K] = expA.astype(bf)
    # shI: identity mapping partitions 0:48 -> out 64:112
    for jj in range(K):
        cb[jj, 384 + HI + jj] = 1.0
    # pat_sum cols 512:514
    cb[0:K, 512] = 1.0
    cb[HI:HI + K, 513] = 1.0
    # ones_red col 514: ones on partitions 64:112
    cb[HI:HI + K, 514] = 1.0
    cf = np.zeros((128, 2), dtype=np.float32)
    cf[0:K, 0] = np.exp(start_transitions.astype(np.float64))
    cf[HI:HI + K, 1] = np.exp(end_transitions.astype(np.float64))
    patbc = np.zeros((2, 128), dtype=np.float32)
    patbc[0, 0:K] = 1.0
    patbc[1, HI:HI + K] = 1.0
    return cb, cf, patbc


def kernel(emissions, transitions, start_transitions, end_transitions,
           tags, mask=None, **_):
    emissions = np.ascontiguousarray(np.asarray(emissions, dtype=np.float32))
    transitions = np.ascontiguousarray(np.asarray(transitions,
                                                  dtype=np.float32))
    start_transitions = np.ascontiguousarray(
        np.asarray(start_transitions, dtype=np.float32))
    end_transitions = np.ascontiguousarray(
        np.asarray(end_transitions, dtype=np.float32))
    tags_i = np.ascontiguousarray(np.asarray(tags).astype(np.int64))

    B, Tt, Kk = emissions.shape
    assert Kk == K and B == N_CORES * BL and Tt == T

    from concourse import bass_utils
    if T not in _NC_CACHE:
        _NC_CACHE[T] = build_nc()
    nc = _NC_CACHE[T]

    cb, cf, patbc = _build_const_arrays(
        transitions, start_transitions, end_transitions)
    in_maps = []
    for c in range(N_CORES):
        sl = slice(c * BL, (c + 1) * BL)
        emT, emR = _prep_core(emissions[sl])
        in_maps.append({
            "emT": emT, "emR": emR,
            "cb": cb, "cf": cf, "patbc": patbc,
        })
    global LAST_RESULT
    res = bass_utils.run_bass_kernel_spmd(nc, in_maps, list(range(N_CORES)),
                                          trace=TRACE)
    LAST_RESULT = res

    b = np.arange(BL)
    logZ_rows = []
    for c in range(N_CORES):
        r = res.results[c]
        out = r["out"].astype(np.float64)
        lnj = out[0, 0:1856]
        led = out[:, 1856:3392]
        logZ = np.zeros(BL)
        for jj in range(NJ):
            logZ += lnj[jj * 64 + b]
        for ii in range(ND):
            logZ -= lnj[(NJ + ii) * 64 + b]
        for s in range(len(NORM_SLOTS)):
            for hh in range(2):
                for blk in range(8):
                    logZ -= led[hh, s * 512 + blk * 64 + b]
        logZ += CSH * (T - 1)
        logZ_rows.append(logZ)
    logZ_rows = np.concatenate(logZ_rows)

    # gold score entirely on host (index gathers over tags)
    em64 = emissions.astype(np.float64)
    gold = np.take_along_axis(em64, tags_i[:, :, None], axis=2)[:, :, 0].sum(1)
    gold += transitions.astype(np.float64)[tags_i[:, :-1], tags_i[:, 1:]].sum(1)
    gold += start_transitions.astype(np.float64)[tags_i[:, 0]]
    gold += end_transitions.astype(np.float64)[tags_i[:, -1]]
    loss = (logZ_rows - gold).mean()
    return np.float32(loss)
